# revision 1
# baseline (speedup 1.0000x reference)
"""Trainium2 Bass kernel for nn_AttnEncoder (attention-gated LSTM encoder).

Math note: in the reference, the softmax attention score is
s[b,d] = (h.wh)[b] + (c.wc)[b] + x_time[b,d] + b_attn, and softmax is taken
over d. The h/c/bias terms are constant along d, so they cancel in softmax:
attn = softmax(x_time) — independent of the recurrence and of t. The model
therefore reduces to an LSTM over w_in_t = attn * x_t with attn computed once.

Layout: everything transposed — features on SBUF partitions, batch on the
free axis. 8-way data parallel over batch (512 batch rows per core).

Per core:
  phase A: DMA x^T tiles [D=128, BC=512] per t; accumulate x_time via DVE STT.
  phase B: softmax over partitions via Exp(ACT) + ones-matmul(PE) column sum +
           reciprocal(DVE) + K=1 broadcast matmul(PE) + multiply.
  phase C: 64 LSTM steps: gates = W_ih.(attn*x_t) + W_hh.h + b in PSUM
           (8 bf16 matmuls), sigmoid/tanh on ACT (fp32, per-gate bias via
           activation bias AP), cell update on DVE in fp32, h in bf16.
"""

import numpy as np
import ml_dtypes

B, T, D, H = 4096, 64, 128, 128
NCORES = 8
BC = B // NCORES          # 512 batch rows per core
G4 = 4 * H                # 512 gate rows
GATE_PERM = [1, 0, 2, 3]  # PSUM gate order [f, i, g, o] from torch [i, f, g, o]

_CACHE = {}


def _legalize_waits(nc, max_waits=1):
    """This container's walrus supports at most one sync wait per instruction.
    Hoist excess waits onto preceding single-wait NoOps on the same engine."""
    import bass_rust

    seq = 0
    for f in nc.m.functions:
        for bb in f.blocks:
            if not any(
                i.sync_info is not None and len(i.sync_info.on_wait) > max_waits
                for i in bb.instructions
            ):
                continue
            new_insts = []
            for inst in bb.instructions:
                si = inst.sync_info
                if si is not None and len(si.on_wait) > max_waits:
                    waits = list(si.on_wait)
                    for w in waits[:-max_waits]:
                        seq += 1
                        nop = bass_rust.InstNoOp(
                            name=f"waitsplit-{seq}", engine=inst.engine
                        )
                        nop.sync_info = bass_rust.SyncInfo(on_wait=[w], on_update=[])
                        new_insts.append(nop)
                    inst.sync_info = bass_rust.SyncInfo(
                        on_wait=waits[-max_waits:], on_update=list(si.on_update)
                    )
                new_insts.append(inst)
            bb.instructions = new_insts


def _build_program(repeats=1, steps=T, no_dma_in=False, no_dma_out=False,
                   streams=2, merged_sigma=False):
    import concourse.bass as bass
    import concourse.tile as tile
    from concourse import mybir

    f32 = mybir.dt.float32
    bf16 = mybir.dt.bfloat16
    AF = mybir.ActivationFunctionType
    OP = mybir.AluOpType

    nc = bass.Bass("TRN2", num_devices=NCORES)
    x_d = nc.dram_tensor("x", [T, D, BC], f32, kind="ExternalInput")
    wih_d = nc.dram_tensor("wih", [D, G4], bf16, kind="ExternalInput")
    whh_d = nc.dram_tensor("whh", [H, G4], bf16, kind="ExternalInput")
    bias_d = nc.dram_tensor("bias", [H, 4], f32, kind="ExternalInput")
    biasr_d = nc.dram_tensor("biasr", [1, G4], f32, kind="ExternalInput")
    wt_d = nc.dram_tensor("wt", [H, T], f32, kind="ExternalInput")
    y_d = nc.dram_tensor("y", [T, H, BC], bf16, kind="ExternalOutput")

    with tile.TileContext(nc) as tc:
        with (
            tc.tile_pool(name="const", bufs=1) as const,
            tc.tile_pool(name="work", bufs=2) as work,
            tc.tile_pool(name="state", bufs=2) as state,
        ):
            wih = const.tile([D, G4], bf16)
            nc.sync.dma_start(out=wih[:], in_=wih_d[:])
            whh = const.tile([H, G4], bf16)
            nc.sync.dma_start(out=whh[:], in_=whh_d[:])
            bias = const.tile([H, 4], f32)
            nc.sync.dma_start(out=bias[:], in_=bias_d[:])
            wtt = const.tile([H, T], f32)
            nc.sync.dma_start(out=wtt[:], in_=wt_d[:])
            onesK = const.tile([128, 1], f32)
            nc.vector.memset(onesK[:], 1.0)
            ones1 = const.tile([1, 128], f32)
            nc.vector.memset(ones1[:], 1.0)
            biasr = const.tile([1, G4], f32)
            nc.sync.dma_start(out=biasr[:], in_=biasr_d[:])
            ones_row = const.tile([1, BC], f32)
            nc.vector.memset(ones_row[:], 1.0)

            # resident input, [D, T*BC] fp32 (128 KiB per partition)
            xs = const.tile([D, T * BC], f32)
            for rep in range(repeats):
              if not no_dma_in:
                for t0 in range(0, T, 4):
                    base = x_d[t0 : t0 + 4, :, :]
                    src_ap = bass.AP(
                        tensor=base.tensor,
                        offset=base.offset,
                        ap=[base.ap[1], base.ap[0], base.ap[2]],
                    )
                    nc.sync.dma_start(
                        out=xs[:, t0 * BC : (t0 + 4) * BC], in_=src_ap
                    )
              elif rep == 0:
                nc.vector.memset(xs[:, 0:BC], 0.01)

              # phase A: x_time = sum_t wt[t] * x_t  (ping-pong STT accumulate)
              acc = work.tile([D, BC], f32, tag="acc")
              nc.vector.memset(acc[:], 0.0)
              for t in range(T):
                acc_new = work.tile([D, BC], f32, tag="acc")
                nc.vector.scalar_tensor_tensor(
                    out=acc_new[:],
                    in0=xs[:, t * BC : (t + 1) * BC],
                    scalar=wtt[:, t : t + 1],
                    in1=acc[:],
                    op0=OP.mult,
                    op1=OP.add,
                )
                acc = acc_new

              # phase B: attn = softmax over partition dim of acc
              attn = work.tile([D, BC], bf16, tag="attn")
              e = work.tile([D, BC], f32, tag="e")
              nc.scalar.activation(out=e[:], in_=acc[:], func=AF.Exp)
              with tc.tile_pool(name="psumB", bufs=1, space="PSUM") as pb:
                s = pb.tile([1, BC], f32, tag="colsum")
                nc.tensor.matmul(s[:], onesK[:], e[:], start=True, stop=True)
                rs = work.tile([1, BC], f32, tag="rs")
                nc.vector.reciprocal(out=rs[:], in_=s[:])
                rb = pb.tile([128, BC], f32, tag="bcast")
                nc.tensor.matmul(rb[:], ones1[:], rs[:], start=True, stop=True)
                nc.vector.tensor_tensor(
                    out=attn[:], in0=e[:], in1=rb[:], op=OP.mult
                )

              # phase C: LSTM recurrence, `streams` interleaved batch slices
              SW = BC // streams  # stream width
              h_prev, c_prev = [], []
              for s in range(streams):
                  hp = state.tile([H, SW], bf16, tag=f"h{s}")
                  nc.vector.memset(hp[:], 0.0)
                  cp = state.tile([H, SW], f32, tag=f"c{s}")
                  nc.vector.memset(cp[:], 0.0)
                  h_prev.append(hp)
                  c_prev.append(cp)

              import bass_rust as _br

              # PSUM packing: one gate per bank at streams=1; two gates per
              # bank at streams=2 (start=True clears the whole bank, so only
              # the first gate in each bank sets start, and explicit deps
              # keep the clearing matmul first).
              BK = 512  # fp32 elements per PSUM bank
              if streams == 1:
                  goff = [0, BK, 2 * BK, 3 * BK]
                  pswidth = 4 * BK
              else:
                  # pack [f|i] in bank0 and [o|g] in bank1 so f,i,o form one
                  # contiguous region for a single merged sigmoid ACT
                  goff = [0, SW, BK + SW, BK]
                  pswidth = 2 * BK
              with tc.tile_pool(name="psum", bufs=2, space="PSUM") as psum:
                for t in range(steps):
                  for s in range(streams):
                    lo = t * BC + s * SW
                    ps = psum.tile([128, pswidth], f32, tag=f"gates{s}")
                    w_in = work.tile([D, SW], bf16, tag=f"win{s}")
                    nc.gpsimd.tensor_tensor(
                        out=w_in[:],
                        in0=attn[:, s * SW : (s + 1) * SW],
                        in1=xs[:, lo : lo + SW],
                        op=OP.mult,
                    )
                    ih_mms = {}
                    gate_order = range(4) if streams == 1 else (0, 1, 3, 2)
                    for g in gate_order:
                        mm = nc.tensor.matmul(
                            ps[:, goff[g] : goff[g] + SW],
                            wih[:, g * H : (g + 1) * H],
                            w_in[:],
                            start=(goff[g] % BK == 0),
                            stop=False,
                        )
                        ih_mms[g] = mm
                    if streams > 1:
                        # non-clearing gate must follow its bank's clearer
                        _br.add_dep_helper(
                            ih_mms[1].ins, ih_mms[0].ins, sync=False,
                            reason="bank0 clear order",
                        )
                        _br.add_dep_helper(
                            ih_mms[2].ins, ih_mms[3].ins, sync=False,
                            reason="bank1 clear order",
                        )
                    if streams > 1 and merged_sigma:
                        # f/i/o biases via rank-1 K=1 matmuls (frees the ACT
                        # bias slot so sigmoid(f,i,o) merges into one op);
                        # g's bias rides the tanh ACT below.
                        for g in (0, 1, 3):
                            bm = nc.tensor.matmul(
                                ps[:, goff[g] : goff[g] + SW],
                                biasr[0:1, g * H : (g + 1) * H],
                                ones_row[0:1, 0:SW],
                                start=False,
                                stop=False,
                            )
                            clearer = ih_mms[0] if goff[g] < BK else ih_mms[3]
                            _br.add_dep_helper(
                                bm.ins, clearer.ins, sync=False,
                                reason="bias after bank clear",
                            )
                    for g in range(4):
                        nc.tensor.matmul(
                            ps[:, goff[g] : goff[g] + SW],
                            whh[:, g * H : (g + 1) * H],
                            h_prev[s][:],
                            start=False,
                            stop=True,
                        )
                    if streams > 1 and merged_sigma:
                        # merged sigmoid over contiguous [f|i|o] region
                        sfio = work.tile([H, 3 * SW], f32, tag=f"sfio{s}")
                        nc.scalar.activation(
                            out=sfio[:], in_=ps[:, 0 : 3 * SW], func=AF.Sigmoid,
                        )
                        sf = sfio[:, 0:SW]
                        si = sfio[:, SW : 2 * SW]
                        so = sfio[:, 2 * SW : 3 * SW]
                        tg_t = work.tile([H, SW], f32, tag=f"tg{s}")
                        nc.scalar.activation(
                            out=tg_t[:], in_=ps[:, goff[2] : goff[2] + SW],
                            func=AF.Tanh, bias=bias[:, 2:3],
                        )
                        tg = tg_t[:]
                    else:
                        sf_t = work.tile([H, SW], f32, tag=f"sf{s}")
                        nc.scalar.activation(
                            out=sf_t[:], in_=ps[:, goff[0] : goff[0] + SW],
                            func=AF.Sigmoid, bias=bias[:, 0:1],
                        )
                        sf = sf_t[:]
                        si_t = work.tile([H, SW], f32, tag=f"si{s}")
                        nc.scalar.activation(
                            out=si_t[:], in_=ps[:, goff[1] : goff[1] + SW],
                            func=AF.Sigmoid, bias=bias[:, 1:2],
                        )
                        si = si_t[:]
                        tg_t = work.tile([H, SW], f32, tag=f"tg{s}")
                        nc.scalar.activation(
                            out=tg_t[:], in_=ps[:, goff[2] : goff[2] + SW],
                            func=AF.Tanh, bias=bias[:, 2:3],
                        )
                        tg = tg_t[:]
                        so_t = work.tile([H, SW], f32, tag=f"so{s}")
                        nc.scalar.activation(
                            out=so_t[:], in_=ps[:, goff[3] : goff[3] + SW],
                            func=AF.Sigmoid, bias=bias[:, 3:4],
                        )
                        so = so_t[:]
                    m1 = work.tile([H, SW], f32, tag=f"m1{s}")
                    nc.vector.tensor_tensor(
                        out=m1[:], in0=sf[:], in1=c_prev[s][:], op=OP.mult
                    )
                    m2 = work.tile([H, SW], f32, tag=f"m2{s}")
                    nc.vector.tensor_tensor(
                        out=m2[:], in0=si[:], in1=tg[:], op=OP.mult
                    )
                    c_new = state.tile([H, SW], f32, tag=f"c{s}")
                    nc.vector.tensor_tensor(
                        out=c_new[:], in0=m1[:], in1=m2[:], op=OP.add
                    )
                    tch = work.tile([H, SW], f32, tag=f"tch{s}")
                    nc.scalar.activation(out=tch[:], in_=c_new[:], func=AF.Tanh)
                    h_new = state.tile([H, SW], bf16, tag=f"h{s}")
                    nc.vector.tensor_tensor(
                        out=h_new[:], in0=so[:], in1=tch[:], op=OP.mult
                    )
                    if not no_dma_out:
                        nc.sync.dma_start(
                            out=y_d[t, :, s * SW : (s + 1) * SW], in_=h_new[:]
                        )
                    h_prev[s], c_prev[s] = h_new, c_new

    _legalize_waits(nc)
    return nc


def _make_runner(nc):
    """jit-once sharded executor modeled on bass2jax.run_bass_via_pjrt."""
    import jax
    import jax.core
    from jax.experimental.shard_map import shard_map
    from jax.sharding import Mesh, PartitionSpec
    from concourse import mybir
    from concourse.bass2jax import (
        _bass_exec_p,
        install_neuronx_cc_hook,
        partition_id_tensor,
    )

    install_neuronx_cc_hook()

    partition_name = nc.partition_id_tensor.name if nc.partition_id_tensor else None
    in_names, out_names, out_avals, zero_outs = [], [], [], []
    for alloc in nc.m.functions[0].allocations:
        if not isinstance(alloc, mybir.MemoryLocationSet):
            continue
        name = alloc.memorylocations[0].name
        if alloc.kind == "ExternalInput":
            if name != partition_name:
                in_names.append(name)
        elif alloc.kind == "ExternalOutput":
            shape = tuple(alloc.tensor_shape)
            dtype = mybir.dt.np(alloc.dtype)
            out_names.append(name)
            out_avals.append(jax.core.ShapedArray(shape, dtype))
            zero_outs.append(np.zeros(shape, dtype))
    n_params = len(in_names)
    n_outs = len(out_avals)
    all_in_names = list(in_names) + list(out_names)
    if partition_name is not None:
        all_in_names.append(partition_name)
    donate = tuple(range(n_params, n_params + n_outs))

    def _body(*args):
        operands = list(args)
        if partition_name is not None:
            operands.append(partition_id_tensor())
        outs = _bass_exec_p.bind(
            *operands,
            out_avals=tuple(out_avals),
            in_names=tuple(all_in_names),
            out_names=tuple(out_names),
            lowering_input_output_aliases=(),
            sim_require_finite=True,
            sim_require_nnan=True,
            nc=nc,
        )
        return tuple(outs)

    devices = jax.devices()[:NCORES]
    mesh = Mesh(np.asarray(devices), ("core",))
    in_specs = (PartitionSpec("core"),) * (n_params + n_outs)
    out_specs = (PartitionSpec("core"),) * n_outs
    sharded = jax.jit(
        shard_map(
            _body, mesh=mesh, in_specs=in_specs, out_specs=out_specs,
            check_rep=False,
        ),
        donate_argnums=donate,
        keep_unused=True,
    )

    def run(per_core_inputs):
        """per_core_inputs: list (len NCORES) of dicts name->np array.
        Returns list of dicts name->np array."""
        concat_in = [
            np.concatenate(
                [np.asarray(per_core_inputs[c][n]) for c in range(NCORES)], axis=0
            )
            for n in in_names
        ]
        concat_zeros = [
            np.zeros((NCORES * z.shape[0], *z.shape[1:]), z.dtype) for z in zero_outs
        ]
        out_arrs = sharded(*concat_in, *concat_zeros)
        return [
            {
                n: np.asarray(out_arrs[i]).reshape(NCORES, *out_avals[i].shape)[c]
                for i, n in enumerate(out_names)
            }
            for c in range(NCORES)
        ]

    def _concat_inputs(per_core_inputs):
        return [
            np.concatenate(
                [np.asarray(per_core_inputs[c][n]) for c in range(NCORES)], axis=0
            )
            for n in in_names
        ]

    def make_chain(k):
        """jit-once executor running the bass program k times back-to-back on
        device, chaining each call's y output into the next call's donated
        output buffer (prevents CSE, amortizes dispatch overhead)."""

        def _chain(*args):
            ins = list(args[:n_params])
            outs = list(args[n_params:])
            for _ in range(k):
                operands = ins + outs
                if partition_name is not None:
                    operands = operands + [partition_id_tensor()]
                outs = list(
                    _bass_exec_p.bind(
                        *operands,
                        out_avals=tuple(out_avals),
                        in_names=tuple(all_in_names),
                        out_names=tuple(out_names),
                        lowering_input_output_aliases=(),
                        sim_require_finite=True,
                        sim_require_nnan=True,
                        nc=nc,
                    )
                )
            return tuple(outs)

        return jax.jit(
            shard_map(
                _chain, mesh=mesh, in_specs=in_specs, out_specs=out_specs,
                check_rep=False,
            ),
            donate_argnums=donate,
            keep_unused=True,
        )

    def device_inputs(per_core_inputs):
        import jax as _jax
        from jax.sharding import NamedSharding

        concat_in = _concat_inputs(per_core_inputs)
        shardings = [NamedSharding(mesh, PartitionSpec("core"))] * n_params
        return [
            _jax.device_put(a, s) for a, s in zip(concat_in, shardings)
        ]

    def fresh_zeros():
        return [
            np.zeros((NCORES * z.shape[0], *z.shape[1:]), z.dtype) for z in zero_outs
        ]

    run.in_names = in_names
    run.out_names = out_names
    run.out_avals = out_avals
    run.zero_outs = zero_outs
    run.sharded = sharded
    run.make_chain = make_chain
    run.device_inputs = device_inputs
    run.fresh_zeros = fresh_zeros
    run.mesh = mesh
    return run


def _get_runner():
    if "runner" not in _CACHE:
        nc = _build_program()
        _CACHE["runner"] = _make_runner(nc)
    return _CACHE["runner"]


def _prep_inputs(input_data, W_ih, W_hh, b_ih, b_hh, W_attn, b_attn):
    input_data = np.ascontiguousarray(np.asarray(input_data, dtype=np.float32))
    W_ih = np.asarray(W_ih, dtype=np.float32)
    W_hh = np.asarray(W_hh, dtype=np.float32)
    b = np.asarray(b_ih, dtype=np.float32) + np.asarray(b_hh, dtype=np.float32)
    W_attn = np.asarray(W_attn, dtype=np.float32)

    wih_r = np.ascontiguousarray(
        W_ih.reshape(4, H, D)[GATE_PERM].reshape(G4, D).T
    ).astype(ml_dtypes.bfloat16)
    whh_r = np.ascontiguousarray(
        W_hh.reshape(4, H, H)[GATE_PERM].reshape(G4, H).T
    ).astype(ml_dtypes.bfloat16)
    bias_r = np.ascontiguousarray(b.reshape(4, H)[GATE_PERM].T)  # [H, 4]
    biasr_r = np.ascontiguousarray(b.reshape(4, H)[GATE_PERM].reshape(1, G4))
    wt = W_attn[0, 2 * H :]  # [T]
    wt_rep = np.ascontiguousarray(np.broadcast_to(wt[None, :], (H, T)))

    per_core = []
    for c in range(NCORES):
        xc = np.ascontiguousarray(
            input_data[c * BC : (c + 1) * BC].transpose(1, 2, 0)
        )  # [T, D, BC]
        per_core.append(
            {"x": xc, "wih": wih_r, "whh": whh_r, "bias": bias_r,
             "biasr": biasr_r, "wt": wt_rep}
        )
    return per_core


def _assemble_output(results):
    out = np.empty((B, T, H), dtype=np.float32)
    for c in range(NCORES):
        yc = results[c]["y"]  # [T, H, BC] bf16
        out[c * BC : (c + 1) * BC] = yc.astype(np.float32).transpose(2, 0, 1)
    return out


def kernel(**inputs):
    per_core = _prep_inputs(**inputs)
    run = _get_runner()
    results = run(per_core)
    return _assemble_output(results)



# revision 4
# speedup vs baseline: 1.0391x; 1.0391x over previous
"""Trainium2 Bass kernel for nn_AttnEncoder (attention-gated LSTM encoder).

Math note: in the reference, the softmax attention score is
s[b,d] = (h.wh)[b] + (c.wc)[b] + x_time[b,d] + b_attn, and softmax is taken
over d. The h/c/bias terms are constant along d, so they cancel in softmax:
attn = softmax(x_time) -- independent of the recurrence and of t. The model
therefore reduces to an LSTM over w_in_t = attn * x_t with attn computed once.

Layout: everything transposed -- features on SBUF partitions, batch on the
free axis. 8-way data parallel over batch (512 batch rows per core).

v2 design (engine-balance targeted; all-bf16 elementwise, numpy-validated
rel err ~1.3e-2 < 2e-2 tolerance):
  phase A: x_time = sum_t wt[t]*x_t computed on PE as 64 accumulating
           matmuls with stationary diag(wt[t]) (bf16, 1 cyc/row), PSUM fp32.
           Overlaps the x DMA-in (x is bf16 in SBUF: 64KB/partition).
  phase B: softmax over partitions: Exp(ACT) + ones-matmul column sum +
           reciprocal(DVE) + K=1 broadcast matmul + multiply -> attn bf16.
  phase C: 64 LSTM steps, 2 interleaved batch streams of width 256.
           Per (t): one DVE multiply w_in = attn*x_t [128,512] bf16.
           Per (t, s): PSUM pack [f|i] bank0, [o|g] bank1; 4 ih matmuls +
           3 rank-1 bias matmuls (all bf16) + 4 hh matmuls (bf16);
           ONE merged sigmoid ACT over contiguous [f|i|o], tanh(g) with
           per-partition bias AP, m1=sf*c (DVE bf16 2x), m2=si*tg (GPSIMD),
           c'=m1+m2 (DVE), tanh(c') (ACT), h=so*tch (DVE) -> DMA out.

Predicted per-core engine busy: ACT ~167us (bottleneck), PE ~160us,
DVE ~90us, GPSIMD ~80us, DMA ~55us.
"""

import numpy as np
import ml_dtypes

B, T, D, H = 4096, 64, 128, 128
NCORES = 8
BC = B // NCORES          # 512 batch rows per core
G4 = 4 * H                # 512 gate rows
GATE_PERM = [1, 0, 2, 3]  # PSUM gate order [f, i, g, o] from torch [i, f, g, o]

_CACHE = {}


def _legalize_waits(nc, max_waits=1):
    """This container's walrus supports at most one sync wait per instruction.
    Hoist excess waits onto preceding single-wait NoOps on the same engine."""
    import bass_rust

    seq = 0
    for f in nc.m.functions:
        for bb in f.blocks:
            if not any(
                i.sync_info is not None and len(i.sync_info.on_wait) > max_waits
                for i in bb.instructions
            ):
                continue
            new_insts = []
            for inst in bb.instructions:
                si = inst.sync_info
                if si is not None and len(si.on_wait) > max_waits:
                    waits = list(si.on_wait)
                    for w in waits[:-max_waits]:
                        seq += 1
                        nop = bass_rust.InstNoOp(
                            name=f"waitsplit-{seq}", engine=inst.engine
                        )
                        nop.sync_info = bass_rust.SyncInfo(on_wait=[w], on_update=[])
                        new_insts.append(nop)
                    inst.sync_info = bass_rust.SyncInfo(
                        on_wait=waits[-max_waits:], on_update=list(si.on_update)
                    )
                new_insts.append(inst)
            bb.instructions = new_insts


def _build_program(repeats=1, steps=T, no_dma_in=False, no_dma_out=False,
                   streams=2):
    import concourse.bass as bass
    import concourse.tile as tile
    from concourse import mybir

    f32 = mybir.dt.float32
    bf16 = mybir.dt.bfloat16
    AF = mybir.ActivationFunctionType
    OP = mybir.AluOpType

    nc = bass.Bass("TRN2", num_devices=NCORES)
    x_d = nc.dram_tensor("x", [T, D, BC], bf16, kind="ExternalInput")
    wih_d = nc.dram_tensor("wih", [D, G4], bf16, kind="ExternalInput")
    whh_d = nc.dram_tensor("whh", [H, G4], bf16, kind="ExternalInput")
    bias_d = nc.dram_tensor("bias", [H, 4], f32, kind="ExternalInput")
    biasr_d = nc.dram_tensor("biasr", [1, G4], bf16, kind="ExternalInput")
    wtdiag_d = nc.dram_tensor("wtdiag", [D, T * D], bf16, kind="ExternalInput")
    y_d = nc.dram_tensor("y", [T, H, BC], bf16, kind="ExternalOutput")

    with tile.TileContext(nc) as tc:
        with (
            tc.tile_pool(name="const", bufs=1) as const,
            tc.tile_pool(name="work", bufs=2) as work,
            tc.tile_pool(name="state", bufs=2) as state,
        ):
            wih = const.tile([D, G4], bf16)
            nc.sync.dma_start(out=wih[:], in_=wih_d[:])
            whh = const.tile([H, G4], bf16)
            nc.sync.dma_start(out=whh[:], in_=whh_d[:])
            bias = const.tile([H, 4], f32)
            nc.sync.dma_start(out=bias[:], in_=bias_d[:])
            wtdiag = const.tile([D, T * D], bf16)
            nc.sync.dma_start(out=wtdiag[:], in_=wtdiag_d[:])
            onesK = const.tile([128, 1], bf16)
            nc.vector.memset(onesK[:], 1.0)
            ones1 = const.tile([1, 128], f32)
            nc.vector.memset(ones1[:], 1.0)
            biasr = const.tile([1, G4], bf16)
            nc.sync.dma_start(out=biasr[:], in_=biasr_d[:])
            ones_row = const.tile([1, BC], bf16)
            nc.vector.memset(ones_row[:], 1.0)

            # resident input, [D, T*BC] bf16 (64 KiB per partition)
            xs = const.tile([D, T * BC], bf16)
            for rep in range(repeats):
              if not no_dma_in:
                for t0 in range(0, T, 4):
                    base = x_d[t0 : t0 + 4, :, :]
                    src_ap = bass.AP(
                        tensor=base.tensor,
                        offset=base.offset,
                        ap=[base.ap[1], base.ap[0], base.ap[2]],
                    )
                    nc.sync.dma_start(
                        out=xs[:, t0 * BC : (t0 + 4) * BC], in_=src_ap
                    )
              elif rep == 0:
                nc.vector.memset(xs[:, 0:BC], 0.01)

              # phase A: x_time = sum_t wt[t] * x_t on PE via accumulating
              # matmuls with stationary diag(wt[t]).
              with tc.tile_pool(name="psumA", bufs=1, space="PSUM") as pa:
                acc = pa.tile([D, BC], f32, tag="acc")
                for t in range(T):
                    nc.tensor.matmul(
                        acc[:],
                        wtdiag[:, t * D : (t + 1) * D],
                        xs[:, t * BC : (t + 1) * BC],
                        start=(t == 0),
                        stop=(t == T - 1),
                    )

                # phase B: attn = softmax over partition dim of acc
                attn = work.tile([D, BC], bf16, tag="attn")
                e = work.tile([D, BC], bf16, tag="e")
                nc.scalar.activation(out=e[:], in_=acc[:], func=AF.Exp)
              with tc.tile_pool(name="psumB", bufs=1, space="PSUM") as pb:
                s = pb.tile([1, BC], f32, tag="colsum")
                nc.tensor.matmul(s[:], onesK[:], e[:], start=True, stop=True)
                rs = work.tile([1, BC], f32, tag="rs")
                nc.vector.reciprocal(out=rs[:], in_=s[:])
                rb = pb.tile([128, BC], f32, tag="bcast")
                nc.tensor.matmul(rb[:], ones1[:], rs[:], start=True, stop=True)
                nc.vector.tensor_tensor(
                    out=attn[:], in0=e[:], in1=rb[:], op=OP.mult
                )

              # phase C: LSTM recurrence, `streams` interleaved batch slices
              SW = BC // streams  # stream width
              h_prev, c_prev = [], []
              for s in range(streams):
                  hp = state.tile([H, SW], bf16, tag=f"h{s}")
                  nc.vector.memset(hp[:], 0.0)
                  cp = state.tile([H, SW], bf16, tag=f"c{s}")
                  nc.vector.memset(cp[:], 0.0)
                  h_prev.append(hp)
                  c_prev.append(cp)

              import bass_rust as _br

              # PSUM packing: two gates per bank; [f|i] in bank0 and [o|g]
              # in bank1 so f,i,o form one contiguous region for a single
              # merged sigmoid ACT (start=True clears the whole bank, so
              # only the first gate in each bank sets start, and explicit
              # deps keep the clearing matmul first).
              BK = 512  # fp32 elements per PSUM bank
              goff = [0, SW, BK + SW, BK]  # f, i, g, o
              pswidth = 2 * BK
              with tc.tile_pool(name="psum", bufs=2, space="PSUM") as psum:
                for t in range(steps):
                  w_in = work.tile([D, BC], bf16, tag="win")
                  nc.vector.tensor_tensor(
                      out=w_in[:],
                      in0=attn[:],
                      in1=xs[:, t * BC : (t + 1) * BC],
                      op=OP.mult,
                  )
                  for s in range(streams):
                    ps = psum.tile([128, pswidth], f32, tag=f"gates{s}")
                    ih_mms = {}
                    for g in (0, 1, 3, 2):
                        mm = nc.tensor.matmul(
                            ps[:, goff[g] : goff[g] + SW],
                            wih[:, g * H : (g + 1) * H],
                            w_in[:, s * SW : (s + 1) * SW],
                            start=(goff[g] % BK == 0),
                            stop=False,
                        )
                        ih_mms[g] = mm
                    # non-clearing gate must follow its bank's clearer
                    _br.add_dep_helper(
                        ih_mms[1].ins, ih_mms[0].ins, sync=False,
                        reason="bank0 clear order",
                    )
                    _br.add_dep_helper(
                        ih_mms[2].ins, ih_mms[3].ins, sync=False,
                        reason="bank1 clear order",
                    )
                    # f/i/o biases via rank-1 K=1 bf16 matmuls (frees the ACT
                    # bias slot so sigmoid(f,i,o) merges into one op);
                    # g's bias rides the tanh ACT below.
                    for g in (0, 1, 3):
                        bm = nc.tensor.matmul(
                            ps[:, goff[g] : goff[g] + SW],
                            biasr[0:1, g * H : (g + 1) * H],
                            ones_row[0:1, 0:SW],
                            start=False,
                            stop=False,
                        )
                        clearer = ih_mms[0] if goff[g] < BK else ih_mms[3]
                        _br.add_dep_helper(
                            bm.ins, clearer.ins, sync=False,
                            reason="bias after bank clear",
                        )
                    for g in range(4):
                        nc.tensor.matmul(
                            ps[:, goff[g] : goff[g] + SW],
                            whh[:, g * H : (g + 1) * H],
                            h_prev[s][:],
                            start=False,
                            stop=True,
                        )
                    # merged sigmoid over contiguous [f|i|o] region, bf16 out
                    sfio = work.tile([H, 3 * SW], bf16, tag=f"sfio{s}")
                    nc.scalar.activation(
                        out=sfio[:], in_=ps[:, 0 : 3 * SW], func=AF.Sigmoid,
                    )
                    sf = sfio[:, 0:SW]
                    si = sfio[:, SW : 2 * SW]
                    so = sfio[:, 2 * SW : 3 * SW]
                    tg = work.tile([H, SW], bf16, tag=f"tg{s}")
                    nc.scalar.activation(
                        out=tg[:], in_=ps[:, goff[2] : goff[2] + SW],
                        func=AF.Tanh, bias=bias[:, 2:3],
                    )
                    m1 = work.tile([H, SW], bf16, tag=f"m1{s}")
                    nc.vector.tensor_tensor(
                        out=m1[:], in0=sf, in1=c_prev[s][:], op=OP.mult
                    )
                    m2 = work.tile([H, SW], bf16, tag=f"m2{s}")
                    nc.gpsimd.tensor_tensor(
                        out=m2[:], in0=si, in1=tg[:], op=OP.mult
                    )
                    c_new = state.tile([H, SW], bf16, tag=f"c{s}")
                    nc.vector.tensor_tensor(
                        out=c_new[:], in0=m1[:], in1=m2[:], op=OP.add
                    )
                    tch = work.tile([H, SW], bf16, tag=f"tch{s}")
                    nc.scalar.activation(out=tch[:], in_=c_new[:], func=AF.Tanh)
                    h_new = state.tile([H, SW], bf16, tag=f"h{s}")
                    nc.vector.tensor_tensor(
                        out=h_new[:], in0=so, in1=tch[:], op=OP.mult
                    )
                    if not no_dma_out:
                        nc.sync.dma_start(
                            out=y_d[t, :, s * SW : (s + 1) * SW], in_=h_new[:]
                        )
                    h_prev[s], c_prev[s] = h_new, c_new

    _legalize_waits(nc)
    return nc


def _make_runner(nc):
    """jit-once sharded executor modeled on bass2jax.run_bass_via_pjrt."""
    import jax
    import jax.core
    from jax.experimental.shard_map import shard_map
    from jax.sharding import Mesh, PartitionSpec
    from concourse import mybir
    from concourse.bass2jax import (
        _bass_exec_p,
        install_neuronx_cc_hook,
        partition_id_tensor,
    )

    install_neuronx_cc_hook()

    partition_name = nc.partition_id_tensor.name if nc.partition_id_tensor else None
    in_names, out_names, out_avals, zero_outs = [], [], [], []
    for alloc in nc.m.functions[0].allocations:
        if not isinstance(alloc, mybir.MemoryLocationSet):
            continue
        name = alloc.memorylocations[0].name
        if alloc.kind == "ExternalInput":
            if name != partition_name:
                in_names.append(name)
        elif alloc.kind == "ExternalOutput":
            shape = tuple(alloc.tensor_shape)
            dtype = mybir.dt.np(alloc.dtype)
            out_names.append(name)
            out_avals.append(jax.core.ShapedArray(shape, dtype))
            zero_outs.append(np.zeros(shape, dtype))
    n_params = len(in_names)
    n_outs = len(out_avals)
    all_in_names = list(in_names) + list(out_names)
    if partition_name is not None:
        all_in_names.append(partition_name)
    donate = tuple(range(n_params, n_params + n_outs))

    def _body(*args):
        operands = list(args)
        if partition_name is not None:
            operands.append(partition_id_tensor())
        outs = _bass_exec_p.bind(
            *operands,
            out_avals=tuple(out_avals),
            in_names=tuple(all_in_names),
            out_names=tuple(out_names),
            lowering_input_output_aliases=(),
            sim_require_finite=True,
            sim_require_nnan=True,
            nc=nc,
        )
        return tuple(outs)

    devices = jax.devices()[:NCORES]
    mesh = Mesh(np.asarray(devices), ("core",))
    in_specs = (PartitionSpec("core"),) * (n_params + n_outs)
    out_specs = (PartitionSpec("core"),) * n_outs
    sharded = jax.jit(
        shard_map(
            _body, mesh=mesh, in_specs=in_specs, out_specs=out_specs,
            check_rep=False,
        ),
        donate_argnums=donate,
        keep_unused=True,
    )

    def run(per_core_inputs):
        """per_core_inputs: list (len NCORES) of dicts name->np array.
        Returns list of dicts name->np array."""
        concat_in = [
            np.concatenate(
                [np.asarray(per_core_inputs[c][n]) for c in range(NCORES)], axis=0
            )
            for n in in_names
        ]
        concat_zeros = [
            np.zeros((NCORES * z.shape[0], *z.shape[1:]), z.dtype) for z in zero_outs
        ]
        out_arrs = sharded(*concat_in, *concat_zeros)
        return [
            {
                n: np.asarray(out_arrs[i]).reshape(NCORES, *out_avals[i].shape)[c]
                for i, n in enumerate(out_names)
            }
            for c in range(NCORES)
        ]

    def _concat_inputs(per_core_inputs):
        return [
            np.concatenate(
                [np.asarray(per_core_inputs[c][n]) for c in range(NCORES)], axis=0
            )
            for n in in_names
        ]

    def make_chain(k):
        """jit-once executor running the bass program k times back-to-back on
        device, chaining each call's y output into the next call's donated
        output buffer (prevents CSE, amortizes dispatch overhead)."""

        def _chain(*args):
            ins = list(args[:n_params])
            outs = list(args[n_params:])
            for _ in range(k):
                operands = ins + outs
                if partition_name is not None:
                    operands = operands + [partition_id_tensor()]
                outs = list(
                    _bass_exec_p.bind(
                        *operands,
                        out_avals=tuple(out_avals),
                        in_names=tuple(all_in_names),
                        out_names=tuple(out_names),
                        lowering_input_output_aliases=(),
                        sim_require_finite=True,
                        sim_require_nnan=True,
                        nc=nc,
                    )
                )
            return tuple(outs)

        return jax.jit(
            shard_map(
                _chain, mesh=mesh, in_specs=in_specs, out_specs=out_specs,
                check_rep=False,
            ),
            donate_argnums=donate,
            keep_unused=True,
        )

    def device_inputs(per_core_inputs):
        import jax as _jax
        from jax.sharding import NamedSharding

        concat_in = _concat_inputs(per_core_inputs)
        shardings = [NamedSharding(mesh, PartitionSpec("core"))] * n_params
        return [
            _jax.device_put(a, s) for a, s in zip(concat_in, shardings)
        ]

    def fresh_zeros():
        return [
            np.zeros((NCORES * z.shape[0], *z.shape[1:]), z.dtype) for z in zero_outs
        ]

    run.in_names = in_names
    run.out_names = out_names
    run.out_avals = out_avals
    run.zero_outs = zero_outs
    run.sharded = sharded
    run.make_chain = make_chain
    run.device_inputs = device_inputs
    run.fresh_zeros = fresh_zeros
    run.mesh = mesh
    return run


def _get_runner():
    if "runner" not in _CACHE:
        nc = _build_program()
        _CACHE["runner"] = _make_runner(nc)
    return _CACHE["runner"]


def _prep_inputs(input_data, W_ih, W_hh, b_ih, b_hh, W_attn, b_attn):
    input_data = np.ascontiguousarray(np.asarray(input_data, dtype=np.float32))
    W_ih = np.asarray(W_ih, dtype=np.float32)
    W_hh = np.asarray(W_hh, dtype=np.float32)
    b = np.asarray(b_ih, dtype=np.float32) + np.asarray(b_hh, dtype=np.float32)
    W_attn = np.asarray(W_attn, dtype=np.float32)

    wih_r = np.ascontiguousarray(
        W_ih.reshape(4, H, D)[GATE_PERM].reshape(G4, D).T
    ).astype(ml_dtypes.bfloat16)
    whh_r = np.ascontiguousarray(
        W_hh.reshape(4, H, H)[GATE_PERM].reshape(G4, H).T
    ).astype(ml_dtypes.bfloat16)
    bias_r = np.ascontiguousarray(b.reshape(4, H)[GATE_PERM].T)  # [H, 4]
    biasr_r = np.ascontiguousarray(
        b.reshape(4, H)[GATE_PERM].reshape(1, G4)
    ).astype(ml_dtypes.bfloat16)
    wt = W_attn[0, 2 * H :]  # [T]
    # [D, T, D] block-diagonal: wtdiag[d, t, d] = wt[t]
    wtdiag = np.zeros((D, T, D), dtype=ml_dtypes.bfloat16)
    idx = np.arange(D)
    for t in range(T):
        wtdiag[idx, t, idx] = wt[t].astype(ml_dtypes.bfloat16)
    wtdiag = np.ascontiguousarray(wtdiag.reshape(D, T * D))

    per_core = []
    for c in range(NCORES):
        xc = np.ascontiguousarray(
            input_data[c * BC : (c + 1) * BC].transpose(1, 2, 0)
        ).astype(ml_dtypes.bfloat16)  # [T, D, BC] bf16
        per_core.append(
            {"x": xc, "wih": wih_r, "whh": whh_r, "bias": bias_r,
             "biasr": biasr_r, "wtdiag": wtdiag}
        )
    return per_core


def _assemble_output(results):
    out = np.empty((B, T, H), dtype=np.float32)
    for c in range(NCORES):
        yc = results[c]["y"]  # [T, H, BC] bf16
        out[c * BC : (c + 1) * BC] = yc.astype(np.float32).transpose(2, 0, 1)
    return out


def kernel(**inputs):
    per_core = _prep_inputs(**inputs)
    run = _get_runner()
    results = run(per_core)
    return _assemble_output(results)


# revision 10
# speedup vs baseline: 1.0554x; 1.0157x over previous
"""Trainium2 Bass kernel for nn_AttnEncoder (attention-gated LSTM encoder).

Math note: in the reference, the softmax attention score is
s[b,d] = (h.wh)[b] + (c.wc)[b] + x_time[b,d] + b_attn, and softmax is taken
over d. The h/c/bias terms are constant along d, so they cancel in softmax:
attn = softmax(x_time) -- independent of the recurrence and of t. The model
therefore reduces to an LSTM over w_in_t = attn * x_t with attn computed once.

Layout: everything transposed -- features on SBUF partitions, batch on the
free axis. 8-way data parallel over batch (512 batch rows per core).

v2 design (engine-balance targeted; all-bf16 elementwise, numpy-validated
rel err ~1.3e-2 < 2e-2 tolerance):
  phase A: x_time = sum_t wt[t]*x_t computed on PE as 64 accumulating
           matmuls with stationary diag(wt[t]) (bf16, 1 cyc/row), PSUM fp32.
           Overlaps the x DMA-in (x is bf16 in SBUF: 64KB/partition).
  phase B: softmax over partitions: Exp(ACT) + ones-matmul column sum +
           reciprocal(DVE) + K=1 broadcast matmul + multiply -> attn bf16.
  phase C: 64 LSTM steps, 2 interleaved batch streams of width 256.
           Per (t): one DVE multiply w_in = attn*x_t [128,512] bf16.
           Per (t, s): PSUM pack [f|i] bank0, [o|g] bank1; 4 ih matmuls +
           3 rank-1 bias matmuls (all bf16) + 4 hh matmuls (bf16);
           ONE merged sigmoid ACT over contiguous [f|i|o], tanh(g) with
           per-partition bias AP, m1=sf*c (DVE bf16 2x), m2=si*tg (GPSIMD),
           c'=m1+m2 (DVE), tanh(c') (ACT), h=so*tch (DVE) -> DMA out.

Predicted per-core engine busy: ACT ~167us (bottleneck), PE ~160us,
DVE ~90us, GPSIMD ~80us, DMA ~55us.
"""

import numpy as np
import ml_dtypes

B, T, D, H = 4096, 64, 128, 128
NCORES = 8
BC = B // NCORES          # 512 batch rows per core
G4 = 4 * H                # 512 gate rows
GATE_PERM = [1, 0, 2, 3]  # PSUM gate order [f, i, g, o] from torch [i, f, g, o]

_CACHE = {}


def _legalize_waits(nc, max_waits=1):
    """This container's walrus supports at most one sync wait per instruction.
    Hoist excess waits onto preceding single-wait NoOps on the same engine."""
    import bass_rust

    seq = 0
    for f in nc.m.functions:
        for bb in f.blocks:
            if not any(
                i.sync_info is not None and len(i.sync_info.on_wait) > max_waits
                for i in bb.instructions
            ):
                continue
            new_insts = []
            for inst in bb.instructions:
                si = inst.sync_info
                if si is not None and len(si.on_wait) > max_waits:
                    waits = list(si.on_wait)
                    for w in waits[:-max_waits]:
                        seq += 1
                        nop = bass_rust.InstNoOp(
                            name=f"waitsplit-{seq}", engine=inst.engine
                        )
                        nop.sync_info = bass_rust.SyncInfo(on_wait=[w], on_update=[])
                        new_insts.append(nop)
                    inst.sync_info = bass_rust.SyncInfo(
                        on_wait=waits[-max_waits:], on_update=list(si.on_update)
                    )
                new_insts.append(inst)
            bb.instructions = new_insts


def _build_program(repeats=1, steps=T, no_dma_in=False, no_dma_out=False,
                   streams=2):
    import concourse.bass as bass
    import concourse.tile as tile
    from concourse import mybir

    f32 = mybir.dt.float32
    bf16 = mybir.dt.bfloat16
    AF = mybir.ActivationFunctionType
    OP = mybir.AluOpType

    nc = bass.Bass("TRN2", num_devices=NCORES)
    x_d = nc.dram_tensor("x", [T, D, BC], bf16, kind="ExternalInput")
    wih_d = nc.dram_tensor("wih", [D, G4], bf16, kind="ExternalInput")
    whh_d = nc.dram_tensor("whh", [H, G4], bf16, kind="ExternalInput")
    bias_d = nc.dram_tensor("bias", [H, 4], f32, kind="ExternalInput")
    biasr_d = nc.dram_tensor("biasr", [1, G4], bf16, kind="ExternalInput")
    wtdiag_d = nc.dram_tensor("wtdiag", [D, T * D], bf16, kind="ExternalInput")
    y_d = nc.dram_tensor("y", [T, H, BC], bf16, kind="ExternalOutput")

    with tile.TileContext(nc) as tc:
        with (
            tc.tile_pool(name="const", bufs=1) as const,
            tc.tile_pool(name="work", bufs=3) as work,
            tc.tile_pool(name="state", bufs=2) as state,
        ):
            wih = const.tile([D, G4], bf16)
            nc.sync.dma_start(out=wih[:], in_=wih_d[:])
            whh = const.tile([H, G4], bf16)
            nc.sync.dma_start(out=whh[:], in_=whh_d[:])
            bias = const.tile([H, 4], f32)
            nc.sync.dma_start(out=bias[:], in_=bias_d[:])
            wtdiag = const.tile([D, T * D], bf16)
            nc.sync.dma_start(out=wtdiag[:], in_=wtdiag_d[:])
            onesK = const.tile([128, 1], bf16)
            nc.vector.memset(onesK[:], 1.0)
            ones1 = const.tile([1, 128], f32)
            nc.vector.memset(ones1[:], 1.0)
            biasr = const.tile([1, G4], bf16)
            nc.sync.dma_start(out=biasr[:], in_=biasr_d[:])
            ones_row = const.tile([1, BC], bf16)
            nc.vector.memset(ones_row[:], 1.0)

            # resident input, [D, T*BC] bf16 (64 KiB per partition)
            xs = const.tile([D, T * BC], bf16)
            for rep in range(repeats):
              if not no_dma_in:
                for t0 in range(0, T, 4):
                    base = x_d[t0 : t0 + 4, :, :]
                    src_ap = bass.AP(
                        tensor=base.tensor,
                        offset=base.offset,
                        ap=[base.ap[1], base.ap[0], base.ap[2]],
                    )
                    nc.sync.dma_start(
                        out=xs[:, t0 * BC : (t0 + 4) * BC], in_=src_ap
                    )
              elif rep == 0:
                nc.vector.memset(xs[:, 0:BC], 0.01)

              # phase A: x_time = sum_t wt[t] * x_t on PE via accumulating
              # matmuls with stationary diag(wt[t]).
              with tc.tile_pool(name="psumA", bufs=1, space="PSUM") as pa:
                acc = pa.tile([D, BC], f32, tag="acc")
                for t in range(T):
                    nc.tensor.matmul(
                        acc[:],
                        wtdiag[:, t * D : (t + 1) * D],
                        xs[:, t * BC : (t + 1) * BC],
                        start=(t == 0),
                        stop=(t == T - 1),
                    )

                # phase B: attn = softmax over partition dim of acc
                attn = work.tile([D, BC], bf16, tag="attn")
                e = work.tile([D, BC], bf16, tag="e")
                nc.scalar.activation(out=e[:], in_=acc[:], func=AF.Exp)
              with tc.tile_pool(name="psumB", bufs=1, space="PSUM") as pb:
                s = pb.tile([1, BC], f32, tag="colsum")
                nc.tensor.matmul(s[:], onesK[:], e[:], start=True, stop=True)
                rs = work.tile([1, BC], f32, tag="rs")
                nc.vector.reciprocal(out=rs[:], in_=s[:])
                rb = pb.tile([128, BC], f32, tag="bcast")
                nc.tensor.matmul(rb[:], ones1[:], rs[:], start=True, stop=True)
                nc.vector.tensor_tensor(
                    out=attn[:], in0=e[:], in1=rb[:], op=OP.mult
                )

              # phase C: LSTM recurrence, `streams` interleaved batch slices
              SW = BC // streams  # stream width
              h_prev, c_prev = [], []
              for s in range(streams):
                  hp = state.tile([H, SW], bf16, tag=f"h{s}")
                  nc.vector.memset(hp[:], 0.0)
                  cp = state.tile([H, SW], bf16, tag=f"c{s}")
                  nc.vector.memset(cp[:], 0.0)
                  h_prev.append(hp[:])
                  c_prev.append(cp[:])

              import bass_rust as _br

              # PSUM packing: two gates per bank; [f|i] in bank0 and [o|g]
              # in bank1 so f,i,o form one contiguous region for a single
              # merged sigmoid ACT (start=True clears the whole bank, so
              # only the first gate in each bank sets start, and explicit
              # deps keep the clearing matmul first).
              BK = 512  # fp32 elements per PSUM bank
              goff = [0, SW, BK + SW, BK]  # f, i, g, o
              pswidth = 2 * BK
              with tc.tile_pool(name="psum", bufs=2, space="PSUM") as psum:
                for t in range(steps):
                  w_in = work.tile([D, BC], bf16, tag="win")
                  nc.gpsimd.tensor_tensor(
                      out=w_in[:],
                      in0=attn[:],
                      in1=xs[:, t * BC : (t + 1) * BC],
                      op=OP.mult,
                  )
                  h_out = state.tile([H, BC], bf16, tag="hout")
                  for s in range(streams):
                    ps = psum.tile([128, pswidth], f32, tag=f"gates{s}")
                    ih_mms = {}
                    for g in (0, 1, 3, 2):
                        mm = nc.tensor.matmul(
                            ps[:, goff[g] : goff[g] + SW],
                            wih[:, g * H : (g + 1) * H],
                            w_in[:, s * SW : (s + 1) * SW],
                            start=(goff[g] % BK == 0),
                            stop=False,
                        )
                        ih_mms[g] = mm
                    # non-clearing gate must follow its bank's clearer
                    _br.add_dep_helper(
                        ih_mms[1].ins, ih_mms[0].ins, sync=False,
                        reason="bank0 clear order",
                    )
                    _br.add_dep_helper(
                        ih_mms[2].ins, ih_mms[3].ins, sync=False,
                        reason="bank1 clear order",
                    )
                    # f/i/o biases via rank-1 K=1 bf16 matmuls (frees the ACT
                    # bias slot so sigmoid(f,i,o) merges into one op);
                    # g's bias rides the tanh ACT below.
                    for g in (0, 1, 3):
                        bm = nc.tensor.matmul(
                            ps[:, goff[g] : goff[g] + SW],
                            biasr[0:1, g * H : (g + 1) * H],
                            ones_row[0:1, 0:SW],
                            start=False,
                            stop=False,
                        )
                        clearer = ih_mms[0] if goff[g] < BK else ih_mms[3]
                        _br.add_dep_helper(
                            bm.ins, clearer.ins, sync=False,
                            reason="bias after bank clear",
                        )
                    # hh order f,i,o,g: sigmoid(fio) depends only on the
                    # first three, so it can start while g's matmul runs.
                    for g in (0, 1, 3, 2):
                        nc.tensor.matmul(
                            ps[:, goff[g] : goff[g] + SW],
                            whh[:, g * H : (g + 1) * H],
                            h_prev[s],
                            start=False,
                            stop=True,
                        )
                    # merged sigmoid over contiguous [f|i|o] region, bf16 out
                    sfio = work.tile([H, 3 * SW], bf16, tag=f"sfio{s}")
                    nc.scalar.activation(
                        out=sfio[:], in_=ps[:, 0 : 3 * SW], func=AF.Sigmoid,
                    )
                    sf = sfio[:, 0:SW]
                    si = sfio[:, SW : 2 * SW]
                    so = sfio[:, 2 * SW : 3 * SW]
                    tg = work.tile([H, SW], bf16, tag=f"tg{s}")
                    nc.scalar.activation(
                        out=tg[:], in_=ps[:, goff[2] : goff[2] + SW],
                        func=AF.Tanh, bias=bias[:, 2:3],
                    )
                    m1 = work.tile([H, SW], bf16, tag=f"m1{s}")
                    nc.vector.tensor_tensor(
                        out=m1[:], in0=sf, in1=c_prev[s], op=OP.mult
                    )
                    m2 = work.tile([H, SW], bf16, tag=f"m2{s}")
                    nc.vector.tensor_tensor(
                        out=m2[:], in0=si, in1=tg[:], op=OP.mult
                    )
                    c_new = state.tile([H, SW], bf16, tag=f"c{s}")
                    nc.vector.tensor_tensor(
                        out=c_new[:], in0=m1[:], in1=m2[:], op=OP.add
                    )
                    tch = work.tile([H, SW], bf16, tag=f"tch{s}")
                    nc.scalar.activation(out=tch[:], in_=c_new[:], func=AF.Tanh)
                    h_new = h_out[:, s * SW : (s + 1) * SW]
                    nc.vector.tensor_tensor(
                        out=h_new, in0=so, in1=tch[:], op=OP.mult
                    )
                    h_prev[s], c_prev[s] = h_new, c_new[:]
                  if not no_dma_out:
                    nc.sync.dma_start(out=y_d[t, :, :], in_=h_out[:])

    _legalize_waits(nc)
    return nc


def _make_runner(nc):
    """jit-once sharded executor modeled on bass2jax.run_bass_via_pjrt."""
    import jax
    import jax.core
    from jax.experimental.shard_map import shard_map
    from jax.sharding import Mesh, PartitionSpec
    from concourse import mybir
    from concourse.bass2jax import (
        _bass_exec_p,
        install_neuronx_cc_hook,
        partition_id_tensor,
    )

    install_neuronx_cc_hook()

    partition_name = nc.partition_id_tensor.name if nc.partition_id_tensor else None
    in_names, out_names, out_avals, zero_outs = [], [], [], []
    for alloc in nc.m.functions[0].allocations:
        if not isinstance(alloc, mybir.MemoryLocationSet):
            continue
        name = alloc.memorylocations[0].name
        if alloc.kind == "ExternalInput":
            if name != partition_name:
                in_names.append(name)
        elif alloc.kind == "ExternalOutput":
            shape = tuple(alloc.tensor_shape)
            dtype = mybir.dt.np(alloc.dtype)
            out_names.append(name)
            out_avals.append(jax.core.ShapedArray(shape, dtype))
            zero_outs.append(np.zeros(shape, dtype))
    n_params = len(in_names)
    n_outs = len(out_avals)
    all_in_names = list(in_names) + list(out_names)
    if partition_name is not None:
        all_in_names.append(partition_name)
    donate = tuple(range(n_params, n_params + n_outs))

    def _body(*args):
        operands = list(args)
        if partition_name is not None:
            operands.append(partition_id_tensor())
        outs = _bass_exec_p.bind(
            *operands,
            out_avals=tuple(out_avals),
            in_names=tuple(all_in_names),
            out_names=tuple(out_names),
            lowering_input_output_aliases=(),
            sim_require_finite=True,
            sim_require_nnan=True,
            nc=nc,
        )
        return tuple(outs)

    devices = jax.devices()[:NCORES]
    mesh = Mesh(np.asarray(devices), ("core",))
    in_specs = (PartitionSpec("core"),) * (n_params + n_outs)
    out_specs = (PartitionSpec("core"),) * n_outs
    sharded = jax.jit(
        shard_map(
            _body, mesh=mesh, in_specs=in_specs, out_specs=out_specs,
            check_rep=False,
        ),
        donate_argnums=donate,
        keep_unused=True,
    )

    def run(per_core_inputs):
        """per_core_inputs: list (len NCORES) of dicts name->np array.
        Returns list of dicts name->np array."""
        concat_in = [
            np.concatenate(
                [np.asarray(per_core_inputs[c][n]) for c in range(NCORES)], axis=0
            )
            for n in in_names
        ]
        concat_zeros = [
            np.zeros((NCORES * z.shape[0], *z.shape[1:]), z.dtype) for z in zero_outs
        ]
        out_arrs = sharded(*concat_in, *concat_zeros)
        return [
            {
                n: np.asarray(out_arrs[i]).reshape(NCORES, *out_avals[i].shape)[c]
                for i, n in enumerate(out_names)
            }
            for c in range(NCORES)
        ]

    def _concat_inputs(per_core_inputs):
        return [
            np.concatenate(
                [np.asarray(per_core_inputs[c][n]) for c in range(NCORES)], axis=0
            )
            for n in in_names
        ]

    def make_chain(k):
        """jit-once executor running the bass program k times back-to-back on
        device, chaining each call's y output into the next call's donated
        output buffer (prevents CSE, amortizes dispatch overhead)."""

        def _chain(*args):
            ins = list(args[:n_params])
            outs = list(args[n_params:])
            for _ in range(k):
                operands = ins + outs
                if partition_name is not None:
                    operands = operands + [partition_id_tensor()]
                outs = list(
                    _bass_exec_p.bind(
                        *operands,
                        out_avals=tuple(out_avals),
                        in_names=tuple(all_in_names),
                        out_names=tuple(out_names),
                        lowering_input_output_aliases=(),
                        sim_require_finite=True,
                        sim_require_nnan=True,
                        nc=nc,
                    )
                )
            return tuple(outs)

        return jax.jit(
            shard_map(
                _chain, mesh=mesh, in_specs=in_specs, out_specs=out_specs,
                check_rep=False,
            ),
            donate_argnums=donate,
            keep_unused=True,
        )

    def device_inputs(per_core_inputs):
        import jax as _jax
        from jax.sharding import NamedSharding

        concat_in = _concat_inputs(per_core_inputs)
        shardings = [NamedSharding(mesh, PartitionSpec("core"))] * n_params
        return [
            _jax.device_put(a, s) for a, s in zip(concat_in, shardings)
        ]

    def fresh_zeros():
        return [
            np.zeros((NCORES * z.shape[0], *z.shape[1:]), z.dtype) for z in zero_outs
        ]

    run.in_names = in_names
    run.out_names = out_names
    run.out_avals = out_avals
    run.zero_outs = zero_outs
    run.sharded = sharded
    run.make_chain = make_chain
    run.device_inputs = device_inputs
    run.fresh_zeros = fresh_zeros
    run.mesh = mesh
    return run


def _get_runner():
    if "runner" not in _CACHE:
        nc = _build_program()
        _CACHE["runner"] = _make_runner(nc)
    return _CACHE["runner"]


def _prep_inputs(input_data, W_ih, W_hh, b_ih, b_hh, W_attn, b_attn):
    input_data = np.ascontiguousarray(np.asarray(input_data, dtype=np.float32))
    W_ih = np.asarray(W_ih, dtype=np.float32)
    W_hh = np.asarray(W_hh, dtype=np.float32)
    b = np.asarray(b_ih, dtype=np.float32) + np.asarray(b_hh, dtype=np.float32)
    W_attn = np.asarray(W_attn, dtype=np.float32)

    wih_r = np.ascontiguousarray(
        W_ih.reshape(4, H, D)[GATE_PERM].reshape(G4, D).T
    ).astype(ml_dtypes.bfloat16)
    whh_r = np.ascontiguousarray(
        W_hh.reshape(4, H, H)[GATE_PERM].reshape(G4, H).T
    ).astype(ml_dtypes.bfloat16)
    bias_r = np.ascontiguousarray(b.reshape(4, H)[GATE_PERM].T)  # [H, 4]
    biasr_r = np.ascontiguousarray(
        b.reshape(4, H)[GATE_PERM].reshape(1, G4)
    ).astype(ml_dtypes.bfloat16)
    wt = W_attn[0, 2 * H :]  # [T]
    # [D, T, D] block-diagonal: wtdiag[d, t, d] = wt[t]
    wtdiag = np.zeros((D, T, D), dtype=ml_dtypes.bfloat16)
    idx = np.arange(D)
    for t in range(T):
        wtdiag[idx, t, idx] = wt[t].astype(ml_dtypes.bfloat16)
    wtdiag = np.ascontiguousarray(wtdiag.reshape(D, T * D))

    per_core = []
    for c in range(NCORES):
        xc = np.ascontiguousarray(
            input_data[c * BC : (c + 1) * BC].transpose(1, 2, 0)
        ).astype(ml_dtypes.bfloat16)  # [T, D, BC] bf16
        per_core.append(
            {"x": xc, "wih": wih_r, "whh": whh_r, "bias": bias_r,
             "biasr": biasr_r, "wtdiag": wtdiag}
        )
    return per_core


def _assemble_output(results):
    out = np.empty((B, T, H), dtype=np.float32)
    for c in range(NCORES):
        yc = results[c]["y"]  # [T, H, BC] bf16
        out[c * BC : (c + 1) * BC] = yc.astype(np.float32).transpose(2, 0, 1)
    return out


def kernel(**inputs):
    per_core = _prep_inputs(**inputs)
    run = _get_runner()
    results = run(per_core)
    return _assemble_output(results)
